# revision 7
# baseline (speedup 1.0000x reference)
"""DECOLLE network (2 CUBA-LIF layers + linear readouts) on 8 trn2 NeuronCores.

Sharding: data-parallel over batch (32 -> 4 per core), weights replicated.
Per core pipeline over time chunks:
  z1 = W1 @ spike        (PE, fp32 matmul, K=f contraction)
  cur1 = leaky-int(z1)   (DVE tensor_tensor_scan along t, one instr per (hc,b))
  v1 loop over t         (2 dependent scalar_tensor_tensor per step, state [128,8,4])
  s1 = v1 >= thresh      (bulk tensor_scalar)
  z2 = W2 @ s1, cur2, v2, s2  (same)
  r1 = R1 @ s1, r2 = R2 @ s2  (PE, accumulated over h chunks, at the end)
"""

import sys

sys.path.insert(0, "/opt/trn_rl_repo")

import numpy as np

import concourse.bass as bass
import concourse.tile as tile
from concourse import bacc, mybir
from concourse.bass_utils import run_bass_kernel_spmd

F32 = mybir.dt.float32
OP = mybir.AluOpType

# Model constants (from the DECOLLE reference)
THRESH = 1.25
CUR_DECAY = 0.25   # cur = (1-0.25)*cur + z
VOLT_DECAY = 0.03  # v = (1-0.03)*v + cur

# Shapes (full problem)
B, F, T = 32, 512, 256
H, OUT = 1024, 128
N_CORES = 8
BL = B // N_CORES          # 4 batches per core
TC = 64                    # time chunk
NCH = T // TC              # 4 chunks
HC = H // 128              # 8 h chunks
FC = F // 128              # 4 f chunks


def build_nc():
    nc = bacc.Bacc(None)

    spike = nc.declare_dram_parameter("spike", [BL, F, T], F32, isOutput=False)
    w1t = nc.declare_dram_parameter("W1T", [F, H], F32, isOutput=False)
    w2t = nc.declare_dram_parameter("W2T", [H, H], F32, isOutput=False)
    r1t = nc.declare_dram_parameter("R1T", [H, OUT], F32, isOutput=False)
    r2t = nc.declare_dram_parameter("R2T", [H, OUT], F32, isOutput=False)

    s1_d = nc.declare_dram_parameter("s1", [BL, H, T], F32, isOutput=True)
    v1_d = nc.declare_dram_parameter("v1", [BL, H, T], F32, isOutput=True)
    s2_d = nc.declare_dram_parameter("s2", [BL, H, T], F32, isOutput=True)
    v2_d = nc.declare_dram_parameter("v2", [BL, H, T], F32, isOutput=True)
    r1_d = nc.declare_dram_parameter("r1", [BL, OUT, T], F32, isOutput=True)
    r2_d = nc.declare_dram_parameter("r2", [BL, OUT, T], F32, isOutput=True)

    # DRAM views: partition dim = h (or o), then free (hc-chunk, b, t)
    spike_r = spike[:].rearrange("b (fc p) t -> fc p b t", p=128)
    s1_r = s1_d[:].rearrange("b (hc p) t -> p hc b t", p=128)
    v1_r = v1_d[:].rearrange("b (hc p) t -> p hc b t", p=128)
    s2_r = s2_d[:].rearrange("b (hc p) t -> p hc b t", p=128)
    v2_r = v2_d[:].rearrange("b (hc p) t -> p hc b t", p=128)
    r1_r = r1_d[:].rearrange("b o t -> o b t")
    r2_r = r2_d[:].rearrange("b o t -> o b t")
    w1_r = w1t[:].rearrange("(fc p) h -> fc p h", p=128)
    w2_r = w2t[:].rearrange("(kc p) h -> kc p h", p=128)
    r1w_r = r1t[:].rearrange("(kc p) o -> kc p o", p=128)
    r2w_r = r2t[:].rearrange("(kc p) o -> kc p o", p=128)

    with tile.TileContext(nc) as tc:
        with (
            tc.tile_pool(name="wsb", bufs=1) as wsb,
            tc.tile_pool(name="xsb", bufs=2) as xsb,
            tc.tile_pool(name="cursb", bufs=2) as cursb,
            tc.tile_pool(name="vsb", bufs=2) as vsb,
            tc.tile_pool(name="ssb", bufs=1) as ssb,
            tc.tile_pool(name="rsb", bufs=1) as rsb,
            tc.tile_pool(name="psum1", bufs=1, space="PSUM") as psum1,
            tc.tile_pool(name="psum2", bufs=1, space="PSUM") as psum2,
        ):
            # ---- weights ----
            w1 = []
            for fc in range(FC):
                w = wsb.tile([128, H], F32, name=f"w1_{fc}")
                nc.sync.dma_start(out=w, in_=w1_r[fc])
                w1.append(w)
            w2 = []
            for kc in range(HC):
                w = wsb.tile([128, H], F32, name=f"w2_{kc}")
                nc.sync.dma_start(out=w, in_=w2_r[kc])
                w2.append(w)
            rw1, rw2 = [], []
            for kc in range(HC):
                a = wsb.tile([128, OUT], F32, name=f"rw1_{kc}")
                nc.sync.dma_start(out=a, in_=r1w_r[kc])
                rw1.append(a)
                b_ = wsb.tile([128, OUT], F32, name=f"rw2_{kc}")
                nc.sync.dma_start(out=b_, in_=r2w_r[kc])
                rw2.append(b_)

            c075 = wsb.tile([128, TC], F32, name="c075")
            nc.vector.memset(c075, 1.0 - CUR_DECAY)

            # persistent voltage states
            v1st = wsb.tile([128, HC, BL], F32, name="v1st")
            nc.vector.memset(v1st, 0.0)
            v2st = wsb.tile([128, HC, BL], F32, name="v2st")
            nc.vector.memset(v2st, 0.0)

            # full spike maps stay in SBUF (feed z2 + readouts + one DMA out)
            s1 = ssb.tile([128, HC, BL, T], F32, name="s1sb")
            s2 = ssb.tile([128, HC, BL, T], F32, name="s2sb")
            r1sb = rsb.tile([128, BL, T], F32, name="r1sb")
            r2sb = rsb.tile([128, BL, T], F32, name="r2sb")

            prev_cur1 = None
            prev_cur2 = None
            for c in range(NCH):
                tsl = slice(c * TC, (c + 1) * TC)

                # ---- load spike chunk ----
                x_c = []
                for fc in range(FC):
                    x = xsb.tile([128, BL, TC], F32, tag=f"x{fc}", name=f"x{fc}_{c}")
                    nc.sync.dma_start(out=x, in_=spike_r[fc][:, :, tsl])
                    x_c.append(x)

                # ---- layer 1 GEMM: z1[h, b, t] = sum_f W1T[f, h] * x[f, b, t] ----
                z1p = []
                for g in range(HC // 2):
                    zp = psum1.tile([128, 2, BL, TC], F32, tag=f"g{g}", name=f"z1p{g}_{c}")
                    z1p.append(zp)
                for g in range(HC // 2):
                    for i in range(2):
                        hc = 2 * g + i
                        for fc in range(FC):
                            nc.tensor.matmul(
                                z1p[g][:, i],
                                w1[fc][:, hc * 128:(hc + 1) * 128],
                                x_c[fc],
                                start=(fc == 0),
                                stop=(fc == FC - 1),
                            )

                # ---- current scan L1 (along t, per (hc, b)) ----
                cur1 = cursb.tile([128, HC, BL, TC], F32, tag="cur1", name=f"cur1_{c}")
                for hc in range(HC):
                    g, i = hc // 2, hc % 2
                    for b in range(BL):
                        init = 0.0 if c == 0 else prev_cur1[:, hc, b, TC - 1:TC]
                        nc.vector.tensor_tensor_scan(
                            cur1[:, hc, b, :],
                            c075,
                            z1p[g][:, i, b, :],
                            init,
                            OP.mult,
                            OP.add,
                        )

                # ---- voltage loop L1 ----
                v1o = vsb.tile([128, HC, BL, TC], F32, tag="v1o", name=f"v1o_{c}")
                for t in range(TC):
                    nc.vector.scalar_tensor_tensor(
                        v1o[:, :, :, t], v1st, 1.0 - VOLT_DECAY, cur1[:, :, :, t],
                        OP.mult, OP.add,
                    )
                    nc.vector.scalar_tensor_tensor(
                        v1st, v1o[:, :, :, t], THRESH, v1o[:, :, :, t],
                        OP.is_lt, OP.mult,
                    )

                # ---- spikes L1 + DMA v1 ----
                nc.vector.tensor_scalar(s1[:, :, :, tsl], v1o, THRESH, None, OP.is_ge)
                for hc in range(HC):
                    nc.sync.dma_start(out=v1_r[:, hc, :, tsl], in_=v1o[:, hc])

                # ---- layer 2 GEMM: z2[m, b, t] = sum_h W2T[h, m] * s1[h, b, t] ----
                z2p = []
                for g in range(HC // 2):
                    zp = psum2.tile([128, 2, BL, TC], F32, tag=f"h{g}", name=f"z2p{g}_{c}")
                    z2p.append(zp)
                for g in range(HC // 2):
                    for i in range(2):
                        mc = 2 * g + i
                        for kc in range(HC):
                            nc.tensor.matmul(
                                z2p[g][:, i],
                                w2[kc][:, mc * 128:(mc + 1) * 128],
                                s1[:, kc, :, tsl],
                                start=(kc == 0),
                                stop=(kc == HC - 1),
                            )

                # ---- current scan L2 ----
                cur2 = cursb.tile([128, HC, BL, TC], F32, tag="cur2", name=f"cur2_{c}")
                for hc in range(HC):
                    g, i = hc // 2, hc % 2
                    for b in range(BL):
                        init = 0.0 if c == 0 else prev_cur2[:, hc, b, TC - 1:TC]
                        nc.vector.tensor_tensor_scan(
                            cur2[:, hc, b, :],
                            c075,
                            z2p[g][:, i, b, :],
                            init,
                            OP.mult,
                            OP.add,
                        )

                # ---- voltage loop L2 ----
                v2o = vsb.tile([128, HC, BL, TC], F32, tag="v2o", name=f"v2o_{c}")
                for t in range(TC):
                    nc.vector.scalar_tensor_tensor(
                        v2o[:, :, :, t], v2st, 1.0 - VOLT_DECAY, cur2[:, :, :, t],
                        OP.mult, OP.add,
                    )
                    nc.vector.scalar_tensor_tensor(
                        v2st, v2o[:, :, :, t], THRESH, v2o[:, :, :, t],
                        OP.is_lt, OP.mult,
                    )

                # ---- spikes L2 + DMA v2 ----
                nc.vector.tensor_scalar(s2[:, :, :, tsl], v2o, THRESH, None, OP.is_ge)
                for hc in range(HC):
                    nc.sync.dma_start(out=v2_r[:, hc, :, tsl], in_=v2o[:, hc])

                prev_cur1 = cur1
                prev_cur2 = cur2

            # ---- spike DMA out ----
            for hc in range(HC):
                nc.sync.dma_start(out=s1_r[:, hc], in_=s1[:, hc])
                nc.sync.dma_start(out=s2_r[:, hc], in_=s2[:, hc])

            # ---- readouts: r[o, b, t] = sum_h RT[h, o] * s[h, b, t] ----
            for c in range(NCH):
                tsl = slice(c * TC, (c + 1) * TC)
                rp = psum1.tile([128, 2, BL, TC], F32, tag="g0", name=f"rp_{c}")
                for kc in range(HC):
                    nc.tensor.matmul(
                        rp[:, 0], rw1[kc], s1[:, kc, :, tsl],
                        start=(kc == 0), stop=(kc == HC - 1),
                    )
                for kc in range(HC):
                    nc.tensor.matmul(
                        rp[:, 1], rw2[kc], s2[:, kc, :, tsl],
                        start=(kc == 0), stop=(kc == HC - 1),
                    )
                nc.scalar.copy(r1sb[:, :, tsl], rp[:, 0])
                nc.scalar.copy(r2sb[:, :, tsl], rp[:, 1])
            nc.sync.dma_start(out=r1_r, in_=r1sb)
            nc.sync.dma_start(out=r2_r, in_=r2sb)

    nc.compile()
    return nc


_NC_CACHE = {}


def _get_nc():
    if "nc" not in _NC_CACHE:
        _NC_CACHE["nc"] = build_nc()
    return _NC_CACHE["nc"]


def run_cores(spike, W1, W2, R1, R2, trace=False):
    nc = _get_nc()
    w1t = np.ascontiguousarray(W1.T).astype(np.float32)
    w2t = np.ascontiguousarray(W2.T).astype(np.float32)
    r1t = np.ascontiguousarray(R1.T).astype(np.float32)
    r2t = np.ascontiguousarray(R2.T).astype(np.float32)
    spike = np.ascontiguousarray(spike).astype(np.float32)
    in_maps = [
        {
            "spike": spike[c * BL:(c + 1) * BL],
            "W1T": w1t,
            "W2T": w2t,
            "R1T": r1t,
            "R2T": r2t,
        }
        for c in range(N_CORES)
    ]
    return run_bass_kernel_spmd(
        nc, in_maps, list(range(N_CORES)), trace=trace
    )


def kernel(spike, W1, W2, R1, R2):
    res = run_cores(spike, W1, W2, R1, R2).results
    s1 = np.concatenate([res[c]["s1"] for c in range(N_CORES)], axis=0)
    v1 = np.concatenate([res[c]["v1"] for c in range(N_CORES)], axis=0)
    s2 = np.concatenate([res[c]["s2"] for c in range(N_CORES)], axis=0)
    v2 = np.concatenate([res[c]["v2"] for c in range(N_CORES)], axis=0)
    r1 = np.concatenate([res[c]["r1"] for c in range(N_CORES)], axis=0)
    r2 = np.concatenate([res[c]["r2"] for c in range(N_CORES)], axis=0)
    c1 = np.float32(s1.mean(dtype=np.float64))
    c2 = np.float32(s2.mean(dtype=np.float64))
    return ((s1, s2), (r1, r2), (v1, v2), (c1, c2))
